# revision 30
# baseline (speedup 1.0000x reference)
"""Mixture-of-Softmaxes with shared embedding — 8-core Trainium2 Bass kernel.

Strategy (tensor-parallel on the vocab output head, per the sharding hint):
  - Vocab dim V is sharded across the 8 cores (Vp = 6283 rows each, zero-padded
    from 50257 to 50264; the 7 pad rows contribute exactly exp(0)=1 to each
    softmax denominator and are corrected by a constant subtraction).
  - The expert transforms (10 experts x 2560x2560) are sharded as 200
    (expert, d-block) jobs, 25 per core, followed by an AllGather of the
    fp8 expert_hidden^T in 5 expert-aligned pieces (expert pairs) so the
    vocab matmuls for early experts start as soon as their piece lands.
  - The big vocab matmul runs in fp8 (e4m3) with DoubleRow perf mode
    (K=256 per instruction): embedding is scaled x64 on the host,
    expert_hidden x16 on chip, undone by the fused exp scale (1/1024).
  - Pass 1 for token-half 0 runs three expert-group sweeps ({0,1}, {2-5},
    {6-9}) gated on the AllGather pieces; half 1 is chunk-outer over all
    experts.  Z is accumulated per (expert, chunk) via the exp
    activation's accum_out, reduced once per half, then AllReduce-ADDed
    across cores (logits are O(1): no max shift needed).
  - Pass 2 (the g/Z-weighted mix + log) for half 0 is emitted interleaved
    into half 1's pass-1 chunk loop so its DVE work hides under the
    tensor-engine stream; half 1's pass 2 is the exposed tail, so three
    experts' multiplies are offloaded to the ACT engine (Copy with
    per-partition scale) with all-bf16 DVE tensor_tensor adds (2x mode).
  - DMA queue discipline (to avoid head-of-line blocking): the SP (sync)
    queue carries only input loads in data-readiness order; spills and
    output stores ride the ACT engine's queue (their producers are ACT
    ops); the Z stats chain and collective enqueues ride GpSimd's queue.
  - RMSNorm is folded in linearly: norm_scale is folded into the expert and
    gate weights on the host; the per-token 1/rms factor is computed on-chip
    in fp32 and applied during the PSUM->SBUF copy of expert_hidden.

kernel(**inputs) takes the full unsharded inputs and returns the full
(1, 256, 50257) float32 logits.
"""
import sys

for _p in ("/opt/trn_rl_repo",):
    if _p not in sys.path:
        sys.path.append(_p)

import numpy as np
import ml_dtypes

import concourse.bacc as bacc
import concourse.mybir as mybir
import concourse.tile as tile
from concourse.bass_utils import run_bass_kernel_spmd
from concourse.masks import make_identity

BF16 = ml_dtypes.bfloat16
FP8 = ml_dtypes.float8_e4m3

NCORES = 8
S = 256          # tokens (B*S)
H = 2560         # hidden
E = 10           # experts
V = 50257        # vocab
KB = H // 128    # 20 k-blocks
NJOBS = E * KB   # 200 (expert, d-block) jobs
JPC = NJOBS // NCORES  # 25 jobs per core
AGP = (10, 10, 5)   # jobs per core per AllGather piece (expert-aligned;
AGOFF = (0, 10, 20)  # each piece costs ~20us fixed, so few big pieces win)
# sh0 sweeps: expert range and the AG pieces that must have landed first
SWEEPS = (((0, 4), (0,)), ((4, 10), (1, 2)))
VP = 6283        # per-core vocab slice (8*6283 = 50264)
NPAD = NCORES * VP - V  # 7 zero-pad vocab rows (on the last core)
CHUNK = 512
NCH = (VP + CHUNK - 1) // CHUNK  # 13
CHUNKS = [(i * CHUNK, min(CHUNK, VP - i * CHUNK)) for i in range(NCH)]
NKEEP = 2        # trailing sh1 chunks kept in SBUF (skip spill+reload)
EPS_NORM = 1e-05
EPS_LOG = 1e-10
EMB_SCALE = 64.0
EH_SCALE = 16.0
INV_SCALE = 1.0 / (EMB_SCALE * EH_SCALE)
NACT = 3         # mix experts whose multiplies run on ACT (rest on DVE)

_nc_cache = None


def _job_of(c, slot):
    """Global job id (= e*KB + k) held by core c at slot (0..JPC-1)."""
    for p, (n, off) in enumerate(zip(AGP, AGOFF)):
        if slot < off + n:
            base = NCORES * sum(AGP[:p])
            return base + c * n + (slot - off)
    raise ValueError(slot)


def build_kernel():
    global _nc_cache
    if _nc_cache is not None:
        return _nc_cache
    f32 = mybir.dt.float32
    bf = mybir.dt.bfloat16
    f8 = mybir.dt.float8e4
    u8 = mybir.dt.uint8
    nc = bacc.Bacc("TRN2", target_bir_lowering=False, debug=False, num_devices=NCORES)

    h32 = nc.declare_dram_parameter("h32", [2, 128, H], bf, isOutput=False)
    hT = nc.declare_dram_parameter("hT", [128, KB, S], bf, isOutput=False)
    gw = nc.declare_dram_parameter("gw", [KB, 128, E], bf, isOutput=False)
    wj = nc.declare_dram_parameter("wjobs", [JPC, 128, KB, 128], bf, isOutput=False)
    embT = nc.declare_dram_parameter("embT", [NCH, 128, KB, CHUNK], f8, isOutput=False)
    out = nc.declare_dram_parameter("out", [S, VP], f32, isOutput=True)

    ehl = [
        nc.dram_tensor(f"eh_local{p}", [128, AGP[p], S], f8)
        for p in range(len(AGP))
    ]
    eha = [
        nc.dram_tensor(
            f"eh_all{p}", [NCORES * 128, AGP[p], S], f8, addr_space="Shared"
        )
        for p in range(len(AGP))
    ]
    zl = [nc.dram_tensor(f"zl{sh}", [128, E], f32) for sh in range(2)]
    za = [
        nc.dram_tensor(f"za{sh}", [128, E], f32, addr_space="Shared")
        for sh in range(2)
    ]
    xsp = nc.dram_tensor("xspill", [NCH, 2, 128, E, CHUNK], bf)

    rg = [list(range(NCORES))]

    with tile.TileContext(nc) as tc:
        with (
            tc.tile_pool(name="et", bufs=3) as etp,          # 10KB slots
            tc.tile_pool(name="stream", bufs=4) as strm,     # 10KB slots
            tc.tile_pool(name="xs", bufs=3) as xsp_pool,     # 10KB slots
            tc.tile_pool(name="ehsh", bufs=1) as ehp,        # 10 x 5KB
            tc.tile_pool(name="stage", bufs=3) as stgp,      # 1.25KB slots
            tc.tile_pool(name="acc", bufs=2) as accp,
            tc.tile_pool(name="tmp", bufs=6) as tmpp,
            tc.tile_pool(name="ot", bufs=2) as otp,
            tc.tile_pool(name="persist", bufs=1) as per,
            tc.tile_pool(name="psmall", bufs=4, space="PSUM") as psS,
            tc.tile_pool(name="psbig", bufs=4, space="PSUM") as psC,
        ):
            epsn = per.tile([128, 1], f32, tag="epsn")
            nc.vector.memset(epsn, EPS_NORM)
            epsl = per.tile([128, 1], f32, tag="epsl")
            nc.vector.memset(epsl, EPS_LOG)

            # raw h^T (bf16), split into 4 k-range loads so the first expert
            # matmuls start as soon as their k-blocks land (subtile deps)
            hTr = strm.tile([128, KB, S], bf, tag="stream")
            for q in range(4):
                nc.sync.dma_start(
                    out=hTr[:, 5 * q : 5 * (q + 1), :],
                    in_=hT[:, 5 * q : 5 * (q + 1), :],
                )

            # ---- per-token RMS factors r_s = 1/sqrt(mean(h^2)+eps) ----
            NSG = H // nc.vector.BN_STATS_FMAX
            r = []
            for sh in range(2):
                ht = etp.tile([128, H], bf, tag="et")
                nc.sync.dma_start(out=ht, in_=h32[sh])
                stats = per.tile(
                    [128, NSG, nc.vector.BN_STATS_DIM], f32, tag=f"st{sh}"
                )
                for sg in range(NSG):
                    nc.vector.bn_stats(
                        out=stats[:, sg, :],
                        in_=ht[
                            :,
                            sg * nc.vector.BN_STATS_FMAX : (sg + 1)
                            * nc.vector.BN_STATS_FMAX,
                        ],
                    )
                mv = per.tile([128, nc.vector.BN_AGGR_DIM], f32, tag=f"mv{sh}")
                nc.vector.bn_aggr(out=mv, in_=stats)
                msq = per.tile([128, 1], f32, tag=f"msq{sh}")
                nc.vector.scalar_tensor_tensor(
                    out=msq, in0=mv[:, 0:1], scalar=mv[:, 0:1], in1=mv[:, 1:2],
                    op0=mybir.AluOpType.mult, op1=mybir.AluOpType.add,
                )
                rsd = per.tile([128, 1], f32, tag=f"rsd{sh}")
                nc.scalar.activation(
                    out=rsd, in_=msq, func=mybir.ActivationFunctionType.Sqrt,
                    bias=epsn[:, 0:1],
                )
                rt = per.tile([128, 1], f32, tag=f"r{sh}")
                nc.vector.reciprocal(rt, rsd)
                r.append(rt)

            # broadcast r over partitions via PE (no DRAM roundtrip)
            ident = per.tile([128, 128], f32, tag="ident")
            make_identity(nc, ident)
            rT = psS.tile([1, S], f32, tag="ps_small")
            nc.tensor.transpose(rT[:, 0:128], r[0], ident)
            nc.tensor.transpose(rT[:, 128:256], r[1], ident)
            rTs = per.tile([1, S], f32, tag="rTs")
            nc.vector.tensor_copy(rTs, rT)
            sc1 = per.tile([1, 128], f32, tag="sc1")
            nc.vector.memset(sc1, EH_SCALE)
            rbps = psS.tile([128, S], f32, tag="ps_small")
            nc.tensor.matmul(rbps, sc1, rTs, start=True, stop=True)
            rbc16 = per.tile([128, S], f32, tag="rbc16")
            nc.vector.tensor_copy(rbc16, rbps)

            # ---- expert transform shard: 25 (e, dblk) jobs; r applied in copy
            elbufs = [
                per.tile([128, AGP[p], S], f8, tag=f"elbuf{p}", name=f"elbuf{p}")
                for p in range(len(AGP))
            ]
            for j in range(JPC):
                wjt = strm.tile([128, KB, 128], bf, tag="stream")
                nc.sync.dma_start(out=wjt, in_=wj[j])
                bps = psS.tile([128, S], f32, tag="ps_small")
                for k in range(KB):
                    nc.tensor.matmul(
                        bps, wjt[:, k, :], hTr[:, k, :],
                        start=(k == 0), stop=(k == KB - 1),
                    )
                for p in range(len(AGP)):
                    if AGOFF[p] <= j < AGOFF[p] + AGP[p]:
                        nc.vector.tensor_mul(
                            elbufs[p][:, j - AGOFF[p], :], bps, rbc16
                        )
                        if j == AGOFF[p] + AGP[p] - 1:
                            # store via the ACT queue so a wait here never
                            # stalls the input-load (sync) queue
                            nc.scalar.dma_start(out=ehl[p][:], in_=elbufs[p])
                            if p <= 1:
                                # piece 2's enqueue waits on its ehl data
                                # (ready only at job 24) and would block
                                # piece 0's staging in GpSimd's FIFO, so it
                                # is enqueued after sweep A's stage instead
                                nc.gpsimd.collective_compute(
                                    "AllGather", mybir.AluOpType.bypass,
                                    replica_groups=rg,
                                    ins=[ehl[p][:]], outs=[eha[p][:]],
                                )

            # ---- gate softmax g (no max shift; logits are O(1)) ----
            gw3 = per.tile([128, KB, E], bf, tag="gw3")
            nc.sync.dma_start(out=gw3, in_=gw[:].rearrange("k p e -> p k e"))
            g = []
            for sh in range(2):
                gps = psS.tile([128, E], f32, tag="ps_small")
                for k in range(KB):
                    nc.tensor.matmul(
                        gps,
                        hTr[:, k, sh * 128 : (sh + 1) * 128],
                        gw3[:, k, :],
                        start=(k == 0),
                        stop=(k == KB - 1),
                    )
                ge = per.tile([128, E], f32, tag=f"ge{sh}")
                gsum = per.tile([128, 1], f32, tag=f"gsum{sh}")
                nc.scalar.activation(
                    out=ge, in_=gps, func=mybir.ActivationFunctionType.Exp,
                    scale=r[sh][:, 0:1], accum_out=gsum[:, 0:1],
                )
                grc = per.tile([128, 1], f32, tag=f"grc{sh}")
                nc.vector.reciprocal(grc, gsum)
                gt = per.tile([128, E], f32, tag=f"g{sh}")
                nc.vector.tensor_scalar_mul(gt, ge, grc[:, 0:1])
                g.append(gt)

            # ---- per-expert eh^T SBUF tiles, gathered piece by piece ----
            ehsh = [
                ehp.tile([128, KB, S], f8, tag=f"ehsh{e}", name=f"ehsh{e}")
                for e in range(E)
            ]

            def emit_stage(p, on_gpsimd=False):
                """Stage AG piece p and copy jobs into ehsh.

                Piece 0 rides GpSimd entirely (DMA queue + copy engine, all
                loads issued before all copies): on the sync queue its loads
                would sit behind the whole input stream, and on DVE its
                copies would sit behind the elbuf muls — either way gating
                sweep A ~30us late.  Later pieces use sync + DVE (both free
                by the time those pieces land)."""
                dma_eng = nc.gpsimd if on_gpsimd else nc.sync
                cp_eng = nc.gpsimd if on_gpsimd else nc.vector
                stps = []
                for c in range(NCORES):
                    stp = stgp.tile(
                        [128, AGP[p], S], f8, name=f"st{p}_{c}",
                        tag="stage0" if on_gpsimd else "stage",
                        bufs=NCORES if on_gpsimd else None,
                    )
                    dma_eng.dma_start(
                        out=stp, in_=eha[p][c * 128 : (c + 1) * 128, :, :]
                    )
                    stps.append(stp)
                for c in range(NCORES):
                    for jj in range(AGP[p]):
                        e, k = divmod(_job_of(c, AGOFF[p] + jj), KB)
                        cp_eng.tensor_copy(
                            out=ehsh[e][:, k, :].bitcast(u8),
                            in_=stps[c][:, jj, :].bitcast(u8),
                        )

            zcol = [
                per.tile([128, E, NCH], f32, tag=f"zcol{sh}", name=f"zcol{sh}")
                for sh in range(2)
            ]
            Rts = [None, None]
            kept = {}

            def emit_chunk_pass1(sh, ci, vn, et3, xs3, elo, e):
                cps = psC.tile([128, CHUNK], f32, tag="psC", name=f"ps{sh}_{ci}_{e}")
                for k2 in range(KB // 2):
                    nc.tensor.matmul(
                        cps[:, :vn],
                        ehsh[e][:, 2 * k2 : 2 * k2 + 2,
                                sh * 128 : (sh + 1) * 128],
                        et3[:, 2 * k2 : 2 * k2 + 2, :vn],
                        start=(k2 == 0),
                        stop=(k2 == KB // 2 - 1),
                        perf_mode=mybir.MatmulPerfMode.DoubleRow,
                    )
                nc.scalar.activation(
                    out=xs3[:, e - elo, :vn], in_=cps[:, :vn],
                    func=mybir.ActivationFunctionType.Exp,
                    scale=INV_SCALE,
                    accum_out=zcol[sh][:, e, ci : ci + 1],
                )

            def emit_z_allreduce(sh):
                """Reduce zcol -> Z, AllReduce, pad-fix, R = g / Z.
                zl/zs DMAs ride GpSimd's queue (decoupled from loads)."""
                zacc = per.tile([128, E], f32, tag=f"zacc{sh}", name=f"zacc{sh}")
                for e in range(E):
                    nc.vector.reduce_sum(
                        out=zacc[:, e : e + 1], in_=zcol[sh][:, e, :],
                        axis=mybir.AxisListType.X,
                    )
                nc.gpsimd.dma_start(out=zl[sh][:], in_=zacc)
                nc.gpsimd.collective_compute(
                    "AllReduce", mybir.AluOpType.add, replica_groups=rg,
                    ins=[zl[sh][:]], outs=[za[sh][:]],
                )
                zs = per.tile([128, E], f32, tag=f"zs{sh}", name=f"zs{sh}")
                nc.gpsimd.dma_start(out=zs, in_=za[sh][:])
                nc.vector.tensor_scalar_add(zs, zs, float(-NPAD))
                zrc = per.tile([128, E], f32, tag=f"zrc{sh}", name=f"zrc{sh}")
                nc.vector.reciprocal(zrc, zs)
                Rt = per.tile([128, E], f32, tag=f"R{sh}", name=f"R{sh}")
                nc.vector.tensor_mul(Rt, g[sh], zrc)
                Rts[sh] = Rt

            def emit_pass2_chunk(sh, ci):
                """mix = sum_e R_e*X_e; ln; store (store via ACT queue)."""
                v0, vn = CHUNKS[ci]
                Rt = Rts[sh]
                if (sh, ci) in kept:
                    xt3 = kept.pop((sh, ci))
                else:
                    xt3 = strm.tile(
                        [128, E, CHUNK], bf, tag="stream", name=f"xt{sh}_{ci}"
                    )
                    nc.sync.dma_start(out=xt3, in_=xsp[ci, sh])
                ndve = E - NACT
                accd = accp.tile([128, CHUNK], bf, tag="accd", name=f"ad{sh}_{ci}")
                nc.vector.tensor_scalar_mul(
                    accd[:, :vn], xt3[:, 0, :vn], Rt[:, 0:1]
                )
                for e in range(1, ndve):
                    nc.vector.scalar_tensor_tensor(
                        out=accd[:, :vn], in0=xt3[:, e, :vn],
                        scalar=Rt[:, e : e + 1], in1=accd[:, :vn],
                        op0=mybir.AluOpType.mult, op1=mybir.AluOpType.add,
                    )
                # experts E-NACT..E-1: multiply on ACT (Copy with per-token
                # scale), add on DVE — all-bf16 tensor_tensor adds run in
                # the DVE 2x 16-bit mode (the 3-source fused stt does not)
                for e in range(ndve, E):
                    tm = tmpp.tile([128, CHUNK], bf, tag="tmp",
                                   name=f"tm{sh}_{ci}_{e}")
                    nc.scalar.activation(
                        out=tm[:, :vn], in_=xt3[:, e, :vn],
                        func=mybir.ActivationFunctionType.Copy,
                        scale=Rt[:, e : e + 1],
                    )
                    nc.vector.tensor_add(
                        accd[:, :vn], accd[:, :vn], tm[:, :vn]
                    )
                ot = otp.tile([128, CHUNK], f32, tag="ot", name=f"ot{sh}_{ci}")
                nc.scalar.activation(
                    out=ot[:, :vn], in_=accd[:, :vn],
                    func=mybir.ActivationFunctionType.Ln,
                    bias=epsl[:, 0:1],
                )
                nc.scalar.dma_start(
                    out=out[sh * 128 : (sh + 1) * 128, v0 : v0 + vn],
                    in_=ot[:, :vn],
                )

            # ---- pass 1, half 0: expert sweeps, staged per AG piece ----
            emit_stage(0, on_gpsimd=True)
            # piece 2's deferred enqueue (see the jobs loop) goes here, after
            # piece 0's copies in GpSimd's FIFO
            nc.gpsimd.collective_compute(
                "AllGather", mybir.AluOpType.bypass, replica_groups=rg,
                ins=[ehl[2][:]], outs=[eha[2][:]],
            )
            for (elo, ehi), pieces in SWEEPS:
                for p in pieces:
                    if p > 0:
                        emit_stage(p)
                for ci, (v0, vn) in enumerate(CHUNKS):
                    et3 = etp.tile([128, KB, CHUNK], f8, tag="et",
                                   name=f"et0_{elo}_{ci}")
                    nc.sync.dma_start(out=et3, in_=embT[ci])
                    xs3 = xsp_pool.tile([128, ehi - elo, CHUNK], bf, tag="xs",
                                        name=f"xs0_{elo}_{ci}")
                    for e in range(elo, ehi):
                        emit_chunk_pass1(0, ci, vn, et3, xs3, elo, e)
                    nc.scalar.dma_start(
                        out=xsp[ci, 0, :, elo:ehi, :], in_=xs3
                    )

            emit_z_allreduce(0)

            # ---- pass 1, half 1 (all experts) + interleaved pass 2, half 0
            for ci, (v0, vn) in enumerate(CHUNKS):
                et3 = etp.tile([128, KB, CHUNK], f8, tag="et", name=f"et1_{ci}")
                nc.sync.dma_start(out=et3, in_=embT[ci])
                xs3 = xsp_pool.tile([128, E, CHUNK], bf, tag="xs",
                                    name=f"xs1_{ci}")
                for e in range(E):
                    emit_chunk_pass1(1, ci, vn, et3, xs3, 0, e)
                if ci >= NCH - NKEEP:
                    kept[(1, ci)] = xs3
                else:
                    nc.scalar.dma_start(out=xsp[ci, 1], in_=xs3)
                if ci <= NCH - 3:
                    emit_pass2_chunk(0, ci)

            # AR1 first so its DVE reduce + gpsimd DMA aren't queued behind
            # the remaining interleaved mixes; those two follow, then the tail
            emit_z_allreduce(1)
            emit_pass2_chunk(0, NCH - 2)
            emit_pass2_chunk(0, NCH - 1)

            # ---- pass 2, half 1 (the exposed tail) ----
            for ci in range(NCH):
                emit_pass2_chunk(1, ci)

    nc.compile()
    _nc_cache = nc
    return nc


def prepare_in_maps(inputs):
    h = np.asarray(inputs["hidden_states"], np.float32).reshape(S, H)
    emb = np.asarray(inputs["embedding_matrix"], np.float32)
    ns = np.asarray(inputs["norm_scale"], np.float32)
    W = np.asarray(inputs["expert_weights"], np.float32)
    G = np.asarray(inputs["gate_weight"], np.float32)

    h32 = np.ascontiguousarray(h.reshape(2, 128, H)).astype(BF16)
    # hT[p, k, s] = h[s, k*128+p]
    hTb = np.ascontiguousarray(h.reshape(S, KB, 128).transpose(2, 1, 0)).astype(BF16)
    gwb = np.ascontiguousarray((G * ns[:, None]).reshape(KB, 128, E)).astype(BF16)

    Wn = W * ns[None, :, None]
    # wjobs_all[j = e*KB + dblk, p, k, d] = Wn[e, k*128+p, dblk*128+d]
    Wr = Wn.reshape(E, KB, 128, KB, 128)
    wjobs_all = np.ascontiguousarray(
        Wr.transpose(0, 3, 2, 1, 4).reshape(NJOBS, 128, KB, 128)
    ).astype(BF16)

    VPAD = NCH * CHUNK  # 6656 (layout padding only; compute uses VP)
    embp = np.zeros((NCORES * VP + (VPAD - VP), H), np.float32)
    embp[:V] = emb

    job_order = [
        [(_job_of(c, slot)) for slot in range(JPC)] for c in range(NCORES)
    ]

    in_maps = []
    for c in range(NCORES):
        esl = embp[c * VP : c * VP + VPAD]  # (VPAD, H) with layout pad tail
        # embT_c[ci, p, k, v] = esl[ci*CHUNK+v, k*128+p] * EMB_SCALE
        embT_c = (
            np.ascontiguousarray(
                esl.reshape(NCH, CHUNK, KB, 128).transpose(0, 3, 2, 1)
            )
            * EMB_SCALE
        ).astype(FP8)
        in_maps.append(
            {
                "h32": h32,
                "hT": hTb,
                "gw": gwb,
                "wjobs": np.ascontiguousarray(wjobs_all[job_order[c]]),
                "embT": embT_c,
            }
        )
    return in_maps


def assemble_output(results):
    full = np.concatenate([results[c]["out"] for c in range(NCORES)], axis=1)
    return np.ascontiguousarray(full[:, :V].reshape(1, S, V).astype(np.float32))


def kernel(**inputs):
    nc = build_kernel()
    in_maps = prepare_in_maps(inputs)
    res = run_bass_kernel_spmd(nc, in_maps, list(range(NCORES)))
    return assemble_output(res.results)


# revision 31
# speedup vs baseline: 1.0903x; 1.0903x over previous
"""Mixture-of-Softmaxes with shared embedding — 8-core Trainium2 Bass kernel.

Strategy (tensor-parallel on the vocab output head, per the sharding hint):
  - Vocab dim V is sharded across the 8 cores (Vp = 6283 rows each, zero-padded
    from 50257 to 50264; the 7 pad rows contribute exactly exp(0)=1 to each
    softmax denominator and are corrected by a constant subtraction).
  - The expert transforms (10 experts x 2560x2560) are sharded as 200
    (expert, d-block) jobs, 25 per core, followed by an AllGather of the
    fp8 expert_hidden^T in 5 expert-aligned pieces (expert pairs) so the
    vocab matmuls for early experts start as soon as their piece lands.
  - The big vocab matmul runs in fp8 (e4m3) with DoubleRow perf mode
    (K=256 per instruction): embedding is scaled x64 on the host,
    expert_hidden x16 on chip, undone by the fused exp scale (1/1024).
  - Pass 1 for token-half 0 runs three expert-group sweeps ({0,1}, {2-5},
    {6-9}) gated on the AllGather pieces; half 1 is chunk-outer over all
    experts.  Z is accumulated per (expert, chunk) via the exp
    activation's accum_out, reduced once per half, then AllReduce-ADDed
    across cores (logits are O(1): no max shift needed).
  - Pass 2 (the g/Z-weighted mix + log) for half 0 is emitted interleaved
    into half 1's pass-1 chunk loop so its DVE work hides under the
    tensor-engine stream; half 1's pass 2 is the exposed tail, so three
    experts' multiplies are offloaded to the ACT engine (Copy with
    per-partition scale) with all-bf16 DVE tensor_tensor adds (2x mode).
  - DMA queue discipline (to avoid head-of-line blocking): the SP (sync)
    queue carries only input loads in data-readiness order; spills and
    output stores ride the ACT engine's queue (their producers are ACT
    ops); the Z stats chain and collective enqueues ride GpSimd's queue.
  - RMSNorm is folded in linearly: norm_scale is folded into the expert and
    gate weights on the host; the per-token 1/rms factor is computed on-chip
    in fp32 and applied during the PSUM->SBUF copy of expert_hidden.

kernel(**inputs) takes the full unsharded inputs and returns the full
(1, 256, 50257) float32 logits.
"""
import sys

for _p in ("/opt/trn_rl_repo",):
    if _p not in sys.path:
        sys.path.append(_p)

import numpy as np
import ml_dtypes

import concourse.bacc as bacc
import concourse.mybir as mybir
import concourse.tile as tile
from concourse.bass_utils import run_bass_kernel_spmd
from concourse.masks import make_identity

BF16 = ml_dtypes.bfloat16
FP8 = ml_dtypes.float8_e4m3

NCORES = 8
S = 256          # tokens (B*S)
H = 2560         # hidden
E = 10           # experts
V = 50257        # vocab
KB = H // 128    # 20 k-blocks
NJOBS = E * KB   # 200 (expert, d-block) jobs
JPC = NJOBS // NCORES  # 25 jobs per core
AGP = (10, 10, 5)   # jobs per core per AllGather piece (expert-aligned;
AGOFF = (0, 10, 20)  # each piece costs ~20us fixed, so few big pieces win)
# sh0 sweeps: expert range and the AG pieces that must have landed first
SWEEPS = (((0, 4), (0,)), ((4, 10), (1, 2)))
VP = 6283        # per-core vocab slice (8*6283 = 50264)
NPAD = NCORES * VP - V  # 7 zero-pad vocab rows (on the last core)
CHUNK = 512
NCH = (VP + CHUNK - 1) // CHUNK  # 13
CHUNKS = [(i * CHUNK, min(CHUNK, VP - i * CHUNK)) for i in range(NCH)]
NKEEP = 2        # trailing sh1 chunks kept in SBUF (skip spill+reload)
EPS_NORM = 1e-05
EPS_LOG = 1e-10
EMB_SCALE = 64.0
EH_SCALE = 16.0
INV_SCALE = 1.0 / (EMB_SCALE * EH_SCALE)
NACT = 3         # tail-mix experts offloaded to ACT/GpSimd

_nc_cache = None


def _job_of(c, slot):
    """Global job id (= e*KB + k) held by core c at slot (0..JPC-1)."""
    for p, (n, off) in enumerate(zip(AGP, AGOFF)):
        if slot < off + n:
            base = NCORES * sum(AGP[:p])
            return base + c * n + (slot - off)
    raise ValueError(slot)


def build_kernel():
    global _nc_cache
    if _nc_cache is not None:
        return _nc_cache
    f32 = mybir.dt.float32
    bf = mybir.dt.bfloat16
    f8 = mybir.dt.float8e4
    u8 = mybir.dt.uint8
    nc = bacc.Bacc("TRN2", target_bir_lowering=False, debug=False, num_devices=NCORES)

    h32 = nc.declare_dram_parameter("h32", [2, 128, H], f32, isOutput=False)
    hT = nc.declare_dram_parameter("hT", [128, KB, S], bf, isOutput=False)
    gw = nc.declare_dram_parameter("gw", [KB, 128, E], bf, isOutput=False)
    wj = nc.declare_dram_parameter("wjobs", [JPC, 128, KB, 128], bf, isOutput=False)
    embT = nc.declare_dram_parameter("embT", [NCH, 128, KB, CHUNK], f8, isOutput=False)
    out = nc.declare_dram_parameter("out", [S, VP], f32, isOutput=True)

    ehl = [
        nc.dram_tensor(f"eh_local{p}", [128, AGP[p], S], f8)
        for p in range(len(AGP))
    ]
    eha = [
        nc.dram_tensor(
            f"eh_all{p}", [NCORES * 128, AGP[p], S], f8, addr_space="Shared"
        )
        for p in range(len(AGP))
    ]
    zl = [nc.dram_tensor(f"zl{sh}", [128, E], f32) for sh in range(2)]
    za = [
        nc.dram_tensor(f"za{sh}", [128, E], f32, addr_space="Shared")
        for sh in range(2)
    ]
    xsp = nc.dram_tensor("xspill", [NCH, 2, 128, E, CHUNK], bf)

    rg = [list(range(NCORES))]

    with tile.TileContext(nc) as tc:
        with (
            tc.tile_pool(name="et", bufs=3) as etp,          # 10KB slots
            tc.tile_pool(name="stream", bufs=4) as strm,     # 10KB slots
            tc.tile_pool(name="xs", bufs=3) as xsp_pool,     # 10KB slots
            tc.tile_pool(name="ehsh", bufs=1) as ehp,        # 10 x 5KB
            tc.tile_pool(name="stage", bufs=3) as stgp,      # 1.25KB slots
            tc.tile_pool(name="acc", bufs=2) as accp,
            tc.tile_pool(name="tmp", bufs=4) as tmpp,
            tc.tile_pool(name="ot", bufs=2) as otp,
            tc.tile_pool(name="persist", bufs=1) as per,
            tc.tile_pool(name="psmall", bufs=4, space="PSUM") as psS,
            tc.tile_pool(name="psbig", bufs=4, space="PSUM") as psC,
        ):
            epsn = per.tile([128, 1], f32, tag="epsn")
            nc.vector.memset(epsn, EPS_NORM)
            epsl = per.tile([128, 1], f32, tag="epsl")
            nc.vector.memset(epsl, EPS_LOG)

            # raw h^T (bf16), split into 4 k-range loads so the first expert
            # matmuls start as soon as their k-blocks land (subtile deps)
            hTr = strm.tile([128, KB, S], bf, tag="stream")
            for q in range(4):
                nc.sync.dma_start(
                    out=hTr[:, 5 * q : 5 * (q + 1), :],
                    in_=hT[:, 5 * q : 5 * (q + 1), :],
                )

            # ---- per-token RMS factors r_s = 1/sqrt(mean(h^2)+eps) ----
            NSG = H // nc.vector.BN_STATS_FMAX
            r = []
            for sh in range(2):
                ht = etp.tile([128, H], f32, tag="et")
                nc.sync.dma_start(out=ht, in_=h32[sh])
                stats = per.tile(
                    [128, NSG, nc.vector.BN_STATS_DIM], f32, tag=f"st{sh}"
                )
                for sg in range(NSG):
                    nc.vector.bn_stats(
                        out=stats[:, sg, :],
                        in_=ht[
                            :,
                            sg * nc.vector.BN_STATS_FMAX : (sg + 1)
                            * nc.vector.BN_STATS_FMAX,
                        ],
                    )
                mv = per.tile([128, nc.vector.BN_AGGR_DIM], f32, tag=f"mv{sh}")
                nc.vector.bn_aggr(out=mv, in_=stats)
                msq = per.tile([128, 1], f32, tag=f"msq{sh}")
                nc.vector.scalar_tensor_tensor(
                    out=msq, in0=mv[:, 0:1], scalar=mv[:, 0:1], in1=mv[:, 1:2],
                    op0=mybir.AluOpType.mult, op1=mybir.AluOpType.add,
                )
                rsd = per.tile([128, 1], f32, tag=f"rsd{sh}")
                nc.scalar.activation(
                    out=rsd, in_=msq, func=mybir.ActivationFunctionType.Sqrt,
                    bias=epsn[:, 0:1],
                )
                rt = per.tile([128, 1], f32, tag=f"r{sh}")
                nc.vector.reciprocal(rt, rsd)
                r.append(rt)

            # broadcast r over partitions via PE (no DRAM roundtrip)
            ident = per.tile([128, 128], f32, tag="ident")
            make_identity(nc, ident)
            rT = psS.tile([1, S], f32, tag="ps_small")
            nc.tensor.transpose(rT[:, 0:128], r[0], ident)
            nc.tensor.transpose(rT[:, 128:256], r[1], ident)
            rTs = per.tile([1, S], f32, tag="rTs")
            nc.vector.tensor_copy(rTs, rT)
            sc1 = per.tile([1, 128], f32, tag="sc1")
            nc.vector.memset(sc1, EH_SCALE)
            rbps = psS.tile([128, S], f32, tag="ps_small")
            nc.tensor.matmul(rbps, sc1, rTs, start=True, stop=True)
            rbc16 = per.tile([128, S], f32, tag="rbc16")
            nc.vector.tensor_copy(rbc16, rbps)

            # ---- expert transform shard: 25 (e, dblk) jobs; r applied in copy
            elbufs = [
                per.tile([128, AGP[p], S], f8, tag=f"elbuf{p}", name=f"elbuf{p}")
                for p in range(len(AGP))
            ]
            for j in range(JPC):
                wjt = strm.tile([128, KB, 128], bf, tag="stream")
                nc.sync.dma_start(out=wjt, in_=wj[j])
                bps = psS.tile([128, S], f32, tag="ps_small")
                for k in range(KB):
                    nc.tensor.matmul(
                        bps, wjt[:, k, :], hTr[:, k, :],
                        start=(k == 0), stop=(k == KB - 1),
                    )
                for p in range(len(AGP)):
                    if AGOFF[p] <= j < AGOFF[p] + AGP[p]:
                        nc.vector.tensor_mul(
                            elbufs[p][:, j - AGOFF[p], :], bps, rbc16
                        )
                        if j == AGOFF[p] + AGP[p] - 1:
                            # store via the ACT queue so a wait here never
                            # stalls the input-load (sync) queue
                            nc.scalar.dma_start(out=ehl[p][:], in_=elbufs[p])
                            nc.gpsimd.collective_compute(
                                "AllGather", mybir.AluOpType.bypass,
                                replica_groups=rg,
                                ins=[ehl[p][:]], outs=[eha[p][:]],
                            )

            # ---- gate softmax g (no max shift; logits are O(1)) ----
            gw3 = per.tile([128, KB, E], bf, tag="gw3")
            nc.sync.dma_start(out=gw3, in_=gw[:].rearrange("k p e -> p k e"))
            g = []
            for sh in range(2):
                gps = psS.tile([128, E], f32, tag="ps_small")
                for k in range(KB):
                    nc.tensor.matmul(
                        gps,
                        hTr[:, k, sh * 128 : (sh + 1) * 128],
                        gw3[:, k, :],
                        start=(k == 0),
                        stop=(k == KB - 1),
                    )
                ge = per.tile([128, E], f32, tag=f"ge{sh}")
                gsum = per.tile([128, 1], f32, tag=f"gsum{sh}")
                nc.scalar.activation(
                    out=ge, in_=gps, func=mybir.ActivationFunctionType.Exp,
                    scale=r[sh][:, 0:1], accum_out=gsum[:, 0:1],
                )
                grc = per.tile([128, 1], f32, tag=f"grc{sh}")
                nc.vector.reciprocal(grc, gsum)
                gt = per.tile([128, E], f32, tag=f"g{sh}")
                nc.vector.tensor_scalar_mul(gt, ge, grc[:, 0:1])
                g.append(gt)

            # ---- per-expert eh^T SBUF tiles, gathered piece by piece ----
            ehsh = [
                ehp.tile([128, KB, S], f8, tag=f"ehsh{e}", name=f"ehsh{e}")
                for e in range(E)
            ]

            def emit_stage(p):
                """Stage AG piece p and copy jobs into ehsh (DVE only —
                GpSimd's queue must stay clear for collective enqueues)."""
                for c in range(NCORES):
                    stp = stgp.tile(
                        [128, AGP[p], S], f8, tag="stage", name=f"st{p}_{c}"
                    )
                    nc.sync.dma_start(
                        out=stp, in_=eha[p][c * 128 : (c + 1) * 128, :, :]
                    )
                    for jj in range(AGP[p]):
                        e, k = divmod(_job_of(c, AGOFF[p] + jj), KB)
                        nc.vector.tensor_copy(
                            out=ehsh[e][:, k, :].bitcast(u8),
                            in_=stp[:, jj, :].bitcast(u8),
                        )

            zcol = [
                per.tile([128, E, NCH], f32, tag=f"zcol{sh}", name=f"zcol{sh}")
                for sh in range(2)
            ]
            Rts = [None, None]
            kept = {}

            def emit_chunk_pass1(sh, ci, vn, et3, xs3, elo, e):
                cps = psC.tile([128, CHUNK], f32, tag="psC", name=f"ps{sh}_{ci}_{e}")
                for k2 in range(KB // 2):
                    nc.tensor.matmul(
                        cps[:, :vn],
                        ehsh[e][:, 2 * k2 : 2 * k2 + 2,
                                sh * 128 : (sh + 1) * 128],
                        et3[:, 2 * k2 : 2 * k2 + 2, :vn],
                        start=(k2 == 0),
                        stop=(k2 == KB // 2 - 1),
                        perf_mode=mybir.MatmulPerfMode.DoubleRow,
                    )
                nc.scalar.activation(
                    out=xs3[:, e - elo, :vn], in_=cps[:, :vn],
                    func=mybir.ActivationFunctionType.Exp,
                    scale=INV_SCALE,
                    accum_out=zcol[sh][:, e, ci : ci + 1],
                )

            def emit_z_allreduce(sh):
                """Reduce zcol -> Z, AllReduce, pad-fix, R = g / Z.
                zl/zs DMAs ride GpSimd's queue (decoupled from loads)."""
                zacc = per.tile([128, E], f32, tag=f"zacc{sh}", name=f"zacc{sh}")
                for e in range(E):
                    nc.vector.reduce_sum(
                        out=zacc[:, e : e + 1], in_=zcol[sh][:, e, :],
                        axis=mybir.AxisListType.X,
                    )
                nc.gpsimd.dma_start(out=zl[sh][:], in_=zacc)
                nc.gpsimd.collective_compute(
                    "AllReduce", mybir.AluOpType.add, replica_groups=rg,
                    ins=[zl[sh][:]], outs=[za[sh][:]],
                )
                zs = per.tile([128, E], f32, tag=f"zs{sh}", name=f"zs{sh}")
                nc.gpsimd.dma_start(out=zs, in_=za[sh][:])
                nc.vector.tensor_scalar_add(zs, zs, float(-NPAD))
                zrc = per.tile([128, E], f32, tag=f"zrc{sh}", name=f"zrc{sh}")
                nc.vector.reciprocal(zrc, zs)
                Rt = per.tile([128, E], f32, tag=f"R{sh}", name=f"R{sh}")
                nc.vector.tensor_mul(Rt, g[sh], zrc)
                Rts[sh] = Rt

            def emit_pass2_chunk(sh, ci):
                """mix = sum_e R_e*X_e; ln; store (store via ACT queue)."""
                v0, vn = CHUNKS[ci]
                Rt = Rts[sh]
                if (sh, ci) in kept:
                    xt3 = kept.pop((sh, ci))
                else:
                    xt3 = strm.tile(
                        [128, E, CHUNK], bf, tag="stream", name=f"xt{sh}_{ci}"
                    )
                    nc.sync.dma_start(out=xt3, in_=xsp[ci, sh])
                ndve = E - NACT
                accd = accp.tile([128, CHUNK], bf, tag="accd", name=f"ad{sh}_{ci}")
                nc.vector.tensor_scalar_mul(
                    accd[:, :vn], xt3[:, 0, :vn], Rt[:, 0:1]
                )
                for e in range(1, ndve):
                    nc.vector.scalar_tensor_tensor(
                        out=accd[:, :vn], in0=xt3[:, e, :vn],
                        scalar=Rt[:, e : e + 1], in1=accd[:, :vn],
                        op0=mybir.AluOpType.mult, op1=mybir.AluOpType.add,
                    )
                # experts E-NACT..E-1: multiply on ACT (Copy with per-token
                # scale), add on DVE — all-bf16 tensor_tensor adds are
                # eligible for the DVE 2x 16-bit mode (stt is not)
                for e in range(ndve, E):
                    tm = tmpp.tile([128, CHUNK], bf, tag="tmp",
                                   name=f"tm{sh}_{ci}_{e}")
                    nc.scalar.activation(
                        out=tm[:, :vn], in_=xt3[:, e, :vn],
                        func=mybir.ActivationFunctionType.Copy,
                        scale=Rt[:, e : e + 1],
                    )
                    nc.vector.tensor_add(
                        accd[:, :vn], accd[:, :vn], tm[:, :vn]
                    )
                ot = otp.tile([128, CHUNK], f32, tag="ot", name=f"ot{sh}_{ci}")
                nc.scalar.activation(
                    out=ot[:, :vn], in_=accd[:, :vn],
                    func=mybir.ActivationFunctionType.Ln,
                    bias=epsl[:, 0:1],
                )
                nc.scalar.dma_start(
                    out=out[sh * 128 : (sh + 1) * 128, v0 : v0 + vn],
                    in_=ot[:, :vn],
                )

            # ---- pass 1, half 0: three expert sweeps, staged per AG piece
            for (elo, ehi), pieces in SWEEPS:
                for p in pieces:
                    emit_stage(p)
                for ci, (v0, vn) in enumerate(CHUNKS):
                    et3 = etp.tile([128, KB, CHUNK], f8, tag="et",
                                   name=f"et0_{elo}_{ci}")
                    nc.sync.dma_start(out=et3, in_=embT[ci])
                    xs3 = xsp_pool.tile([128, ehi - elo, CHUNK], bf, tag="xs",
                                        name=f"xs0_{elo}_{ci}")
                    for e in range(elo, ehi):
                        emit_chunk_pass1(0, ci, vn, et3, xs3, elo, e)
                    nc.scalar.dma_start(
                        out=xsp[ci, 0, :, elo:ehi, :], in_=xs3
                    )

            emit_z_allreduce(0)

            # ---- pass 1, half 1 (all experts) + interleaved pass 2, half 0
            for ci, (v0, vn) in enumerate(CHUNKS):
                et3 = etp.tile([128, KB, CHUNK], f8, tag="et", name=f"et1_{ci}")
                nc.sync.dma_start(out=et3, in_=embT[ci])
                xs3 = xsp_pool.tile([128, E, CHUNK], bf, tag="xs",
                                    name=f"xs1_{ci}")
                for e in range(E):
                    emit_chunk_pass1(1, ci, vn, et3, xs3, 0, e)
                if ci >= NCH - NKEEP:
                    kept[(1, ci)] = xs3
                else:
                    nc.scalar.dma_start(out=xsp[ci, 1], in_=xs3)
                if ci <= NCH - 3:
                    emit_pass2_chunk(0, ci)

            # AR1 first so its DVE reduce + gpsimd DMA aren't queued behind
            # the remaining interleaved mixes; those two follow, then the tail
            emit_z_allreduce(1)
            emit_pass2_chunk(0, NCH - 2)
            emit_pass2_chunk(0, NCH - 1)

            # ---- pass 2, half 1 (the exposed tail) ----
            for ci in range(NCH):
                emit_pass2_chunk(1, ci)

    nc.compile()
    _nc_cache = nc
    return nc


def prepare_in_maps(inputs):
    h = np.asarray(inputs["hidden_states"], np.float32).reshape(S, H)
    emb = np.asarray(inputs["embedding_matrix"], np.float32)
    ns = np.asarray(inputs["norm_scale"], np.float32)
    W = np.asarray(inputs["expert_weights"], np.float32)
    G = np.asarray(inputs["gate_weight"], np.float32)

    h32 = np.ascontiguousarray(h.reshape(2, 128, H))
    # hT[p, k, s] = h[s, k*128+p]
    hTb = np.ascontiguousarray(h.reshape(S, KB, 128).transpose(2, 1, 0)).astype(BF16)
    gwb = np.ascontiguousarray((G * ns[:, None]).reshape(KB, 128, E)).astype(BF16)

    Wn = W * ns[None, :, None]
    # wjobs_all[j = e*KB + dblk, p, k, d] = Wn[e, k*128+p, dblk*128+d]
    Wr = Wn.reshape(E, KB, 128, KB, 128)
    wjobs_all = np.ascontiguousarray(
        Wr.transpose(0, 3, 2, 1, 4).reshape(NJOBS, 128, KB, 128)
    ).astype(BF16)

    VPAD = NCH * CHUNK  # 6656 (layout padding only; compute uses VP)
    embp = np.zeros((NCORES * VP + (VPAD - VP), H), np.float32)
    embp[:V] = emb

    job_order = [
        [(_job_of(c, slot)) for slot in range(JPC)] for c in range(NCORES)
    ]

    in_maps = []
    for c in range(NCORES):
        esl = embp[c * VP : c * VP + VPAD]  # (VPAD, H) with layout pad tail
        # embT_c[ci, p, k, v] = esl[ci*CHUNK+v, k*128+p] * EMB_SCALE
        embT_c = (
            np.ascontiguousarray(
                esl.reshape(NCH, CHUNK, KB, 128).transpose(0, 3, 2, 1)
            )
            * EMB_SCALE
        ).astype(FP8)
        in_maps.append(
            {
                "h32": h32,
                "hT": hTb,
                "gw": gwb,
                "wjobs": np.ascontiguousarray(wjobs_all[job_order[c]]),
                "embT": embT_c,
            }
        )
    return in_maps


def assemble_output(results):
    full = np.concatenate([results[c]["out"] for c in range(NCORES)], axis=1)
    return np.ascontiguousarray(full[:, :V].reshape(1, S, V).astype(np.float32))


def kernel(**inputs):
    nc = build_kernel()
    in_maps = prepare_in_maps(inputs)
    res = run_bass_kernel_spmd(nc, in_maps, list(range(NCORES)))
    return assemble_output(res.results)
